# revision 1
# baseline (speedup 1.0000x reference)
"""BiDiTreeLSTM forest kernel for 8 Trainium2 NeuronCores.

Strategy (data-parallel over trees, per the sharding hint):
  - 256 complete binary trees (depth 8, 511 nodes); 32 trees per core.
  - Everything on-device is FEATURE-MAJOR: [128 features on partitions,
    nodes on the free axis].  H == X == 128 exactly fills the partitions.
  - Host pre-permutes each core's nodes into a level-grouped "chunk-local
    split" order: level blocks stored t=8..0; within a level, the children
    of the parents in 512-column chunk j form two adjacent 512-column
    chunks 2j (left) and 2j+1 (right).  Every child/parent gather in both
    propagation passes is then a contiguous column slice, dependencies
    between levels are chunk-local, and tree id == column mod 32 at every
    level (512 % 32 == 0).
  - Bottom-up then top-down level-synchronous ChildSum TreeLSTM per core,
    matmuls in bf16 (fp32 PSUM accumulate), elementwise in bf16 on the
    vector engine (2x mode), gates on the scalar engine.
  - tanh(c)/h of chunk k are deferred into chunk k+1 ("phase2") so the
    scalar engine never stalls on the c chain.
  - h0/c0 are zeros and b_iou_* are zeros by problem spec; the kernel
    exploits that (they do not affect the output otherwise).
Output per core: [128, 64] fp32 = [root h_bu | leaf-mean h_td] feature-major;
host reassembles the [256, 256] result.
"""

import os
import sys

sys.path.insert(0, "/opt/trn_rl_repo")

import ml_dtypes
import numpy as np

import concourse.bass as bass
import concourse.mybir as mybir
import concourse.tile as tile

B = 256
DEPTH = 8
M = 511
H = 128
NCORES = 8
TPC = B // NCORES            # trees per core = 32
NC_NODES = TPC * M           # 16352
CH = 512                     # moving-dim chunk (one fp32 PSUM bank)

NT = {t: TPC * (1 << t) for t in range(DEPTH + 1)}      # cols per level
OFF = {}
_o = 0
for _t in range(DEPTH, -1, -1):                           # level 8 first
    OFF[_t] = _o
    _o += NT[_t]
assert _o == NC_NODES

F32 = mybir.dt.float32
BF16 = mybir.dt.bfloat16
ActF = mybir.ActivationFunctionType

_NC_CACHE = {}


def _split_multi_waits(nc):
    """This container's walrus accepts at most ONE sync wait per
    instruction; Tile attaches several.  Insert single-wait NoOps."""
    n = 0
    for fn in nc.m.functions:
        for bb in fn.blocks:
            insts = bb.instructions
            new = []
            for inst in insts:
                si = inst.sync_info
                if si is not None and si.on_wait and len(si.on_wait) > 1:
                    waits = list(si.on_wait)
                    for j, w in enumerate(waits[:-1]):
                        new.append(mybir.InstNoOp(
                            name=f"{inst.name}_w{j}",
                            sync_info=mybir.SyncInfo(on_wait=[w], on_update=[]),
                            bass_nofuse=True,
                            engine=inst.engine,
                        ))
                        n += 1
                    si.on_wait = [waits[-1]]
                new.append(inst)
            if len(new) != len(insts):
                bb.instructions[:] = new
    return n


def _build_nc():
    nc = bass.Bass("TRN2")

    XT = nc.dram_tensor("XT", [H, NC_NODES], BF16, kind="ExternalInput")
    # all weight matrices packed: [WBU | UBU | UFBU | WTDX | WTDH | UTD | UFTD]
    WPACK = nc.dram_tensor("WPACK", [H, 2176], BF16, kind="ExternalInput")
    BPACK = nc.dram_tensor("BPACK", [H, 2], F32, kind="ExternalInput")
    OUT = nc.dram_tensor("OUT", [H, 64], F32, kind="ExternalOutput")

    with tile.TileContext(nc) as tc:
        with tc.tile_pool(name="persist", bufs=1) as P, \
             tc.tile_pool(name="ws", bufs=3) as W, \
             tc.tile_pool(name="psiou", bufs=2, space="PSUM") as PI, \
             tc.tile_pool(name="psf", bufs=1, space="PSUM") as PF:

            # ---- persistent SBUF ----
            xt = P.tile([H, NC_NODES], BF16)
            wpk = P.tile([H, 2176], BF16)
            bpk = P.tile([H, 2], F32)
            # xt streamed on the SWDGE (gpsimd) queue so its issue cost
            # overlaps the sync-queue weight loads; graded chunks so level-8
            # compute starts as soon as the first columns land
            nc.gpsimd.dma_start(out=xt[:, 0:512], in_=XT[:, 0:512])
            nc.sync.dma_start(out=wpk[:], in_=WPACK[:])
            nc.sync.dma_start(out=bpk[:], in_=BPACK[:])
            for a, b in ((512, 1536), (1536, 4096), (4096, 10240),
                         (10240, NC_NODES)):
                nc.gpsimd.dma_start(out=xt[:, a:b], in_=XT[:, a:b])
            wbu = wpk[:, 0:384]
            ubu = wpk[:, 384:768]
            ufbu = wpk[:, 768:896]
            wtdx = wpk[:, 896:1280]
            wtdh = wpk[:, 1280:1664]
            utd = wpk[:, 1664:2048]
            uftd = wpk[:, 2048:2176]
            bfbu = bpk[:, 0:1]
            bftd = bpk[:, 1:2]

            hbu = P.tile([H, NC_NODES], BF16)     # all bottom-up h
            ca = P.tile([H, 8192], BF16)          # c ping (even levels)
            cb = P.tile([H, 4096], BF16)          # c pong (odd levels)
            ha = P.tile([H, 2048], BF16)          # td h ping (even, t<8)
            hb = P.tile([H, 4096], BF16)          # td h pong (odd)
            slots = P.tile([H, 512], F32)         # leaf-mean partials
            outsb = P.tile([H, 64], F32)

            def cbuf(t):
                return ca if t % 2 == 0 else cb

            def hbuf(t):
                return ha if t % 2 == 0 else hb

            # ---------------- bottom-up ----------------
            # phase2 (tanh(c), h) of chunk k is emitted during chunk k+1;
            # with the chunk-local layout later chunks never need the
            # immediately-preceding chunk's h, so deferral is safe even
            # across level boundaries.
            def bu_phase2(st):
                t, c0, w, G, sio = st
                cdst = cbuf(t)[:, c0: c0 + w]
                tch = W.tile([H, CH], BF16, tag="tc")
                nc.scalar.activation(out=tch[:, :w], in_=cdst,
                                     func=ActF.Tanh)
                nc.vector.tensor_mul(hbu[:, OFF[t] + c0: OFF[t] + c0 + w],
                                     sio[:, G:G + w], tch[:, :w])
                if t == 0:
                    # fp32 root output
                    nc.vector.tensor_mul(outsb[:, 0:32],
                                         sio[:, G:G + w], tch[:, :w])

            pending = None
            for t in range(DEPTH, -1, -1):
                n = NT[t]
                xoff = OFF[t]
                cw = cbuf(t)
                for c0 in range(0, n, CH):
                    w = min(CH, n - c0)
                    # gate k lives at col k*G; start=True exactly on the
                    # first matmul touching each PSUM bank (start clears
                    # bank-wide has_written bits)
                    G = w if w < CH else CH
                    iou = PI.tile([H, 3 * CH], F32, tag="iou")
                    for k in range(3):
                        nc.tensor.matmul(
                            iou[:, k * G: k * G + w],
                            wbu[:, k * H:(k + 1) * H],
                            xt[:, xoff + c0: xoff + c0 + w],
                            start=(k * G) % 512 == 0, stop=(t == DEPTH))
                    if t < DEPTH:
                        # children of parents [c0:c0+w] are the adjacent
                        # chunks [2c0 : 2c0+w] and [2c0+w : 2c0+2w]
                        choff = OFF[t + 1]
                        hl = hbu[:, choff + 2 * c0: choff + 2 * c0 + w]
                        hr = hbu[:, choff + 2 * c0 + w: choff + 2 * c0 + 2 * w]
                        cc = cbuf(t + 1)
                        psf = PF.tile([H, 2 * CH], F32, tag="pf")
                        nc.tensor.matmul(psf[:, 0:w], ufbu, hl,
                                         start=True, stop=True)
                        nc.tensor.matmul(psf[:, G:G + w], ufbu, hr,
                                         start=G % 512 == 0, stop=True)
                        if w == n:
                            # single-chunk level: U@(hl+hr) as two matmuls
                            # (skips the htild hop on the critical chain)
                            for k in range(3):
                                nc.tensor.matmul(
                                    iou[:, k * G: k * G + w],
                                    ubu[:, k * H:(k + 1) * H],
                                    hl, start=False, stop=False)
                                nc.tensor.matmul(
                                    iou[:, k * G: k * G + w],
                                    ubu[:, k * H:(k + 1) * H],
                                    hr, start=False, stop=True)
                        else:
                            htild = W.tile([H, CH], BF16, tag="htild")
                            nc.vector.tensor_add(htild[:, :w], hl, hr)
                            for k in range(3):
                                nc.tensor.matmul(
                                    iou[:, k * G: k * G + w],
                                    ubu[:, k * H:(k + 1) * H],
                                    htild[:, :w],
                                    start=False, stop=True)
                        f = W.tile([H, 2 * CH], BF16, tag="f")
                        nc.scalar.activation(out=f[:, :G + w],
                                             in_=psf[:, :G + w],
                                             func=ActF.Sigmoid, bias=bfbu)
                        # fc over both (contiguous) children, then fold
                        fc2 = W.tile([H, 2 * CH], BF16, tag="fc2")
                        nc.vector.tensor_mul(fc2[:, :2 * w], f[:, :2 * w],
                                             cc[:, 2 * c0: 2 * c0 + 2 * w])
                        cred = W.tile([H, CH], BF16, tag="cred")
                        nc.vector.tensor_add(cred[:, :w], fc2[:, :w],
                                             fc2[:, w:2 * w])
                    sio = W.tile([H, 2 * CH], BF16, tag="sio")
                    nc.scalar.activation(out=sio[:, :G + w],
                                         in_=iou[:, 0:G + w],
                                         func=ActF.Sigmoid)
                    tu = W.tile([H, CH], BF16, tag="tu")
                    nc.scalar.activation(out=tu[:, :w],
                                         in_=iou[:, 2 * G:2 * G + w],
                                         func=ActF.Tanh)
                    cdst = cw[:, c0: c0 + w]
                    if t < DEPTH:
                        t1 = W.tile([H, CH], BF16, tag="t1")
                        nc.vector.tensor_mul(t1[:, :w], sio[:, :w], tu[:, :w])
                        nc.vector.tensor_add(cdst, t1[:, :w], cred[:, :w])
                    else:
                        nc.vector.tensor_mul(cdst, sio[:, :w], tu[:, :w])
                    if pending is not None:
                        bu_phase2(pending)
                    pending = (t, c0, w, G, sio)
                if n <= 2 * CH:
                    # small level: the next level's first chunk reads this
                    # level's last chunk -- deferral would skip the RAW dep
                    bu_phase2(pending)
                    pending = None
            if pending is not None:
                bu_phase2(pending)
            pending = None

            # ---------------- top-down ----------------
            def td_phase2(st):
                t, coff, w, G, sio = st
                cdst = cbuf(t)[:, coff: coff + w]
                tch = W.tile([H, CH], BF16, tag="tc")
                nc.scalar.activation(out=tch[:, :w], in_=cdst,
                                     func=ActF.Tanh)
                if t < DEPTH:
                    nc.vector.tensor_mul(hbuf(t)[:, coff: coff + w],
                                         sio[:, G:G + w], tch[:, :w])
                else:
                    hn = W.tile([H, CH], BF16, tag="hn")
                    nc.vector.tensor_mul(hn[:, :w],
                                         sio[:, G:G + w], tch[:, :w])
                    ci = coff // CH
                    nc.vector.reduce_sum(
                        out=slots[:, ci * 32:(ci + 1) * 32],
                        in_=hn[:, :w].rearrange("p (k t) -> p t k", t=32),
                        axis=mybir.AxisListType.X)

            for t in range(0, DEPTH + 1):
                n = NT[t]
                xoff = OFF[t]
                cw = cbuf(t)
                if t == 0:
                    w = n  # 32
                    G = w
                    iou = PI.tile([H, 3 * CH], F32, tag="iou")
                    for k in range(3):
                        nc.tensor.matmul(iou[:, k * G: k * G + w],
                                         wtdx[:, k * H:(k + 1) * H],
                                         xt[:, xoff: xoff + w],
                                         start=(k == 0), stop=False)
                        nc.tensor.matmul(iou[:, k * G: k * G + w],
                                         wtdh[:, k * H:(k + 1) * H],
                                         hbu[:, xoff: xoff + w],
                                         start=False, stop=True)
                    sio = W.tile([H, 2 * CH], BF16, tag="sio")
                    nc.scalar.activation(out=sio[:, :G + w],
                                         in_=iou[:, 0:G + w],
                                         func=ActF.Sigmoid)
                    tu = W.tile([H, CH], BF16, tag="tu")
                    nc.scalar.activation(out=tu[:, :w],
                                         in_=iou[:, 2 * G:2 * G + w],
                                         func=ActF.Tanh)
                    nc.vector.tensor_mul(cw[:, 0:w], sio[:, :w], tu[:, :w])
                    td_phase2((0, 0, w, G, sio))
                    continue
                half = n // 2
                hp = hbuf(t - 1)
                cp = cbuf(t - 1)
                for p0 in range(0, half, CH):
                    w = min(CH, half - p0)
                    G = w if w < CH else CH
                    psf = PF.tile([H, 2 * CH], F32, tag="pf")
                    nc.tensor.matmul(psf[:, 0:w], uftd,
                                     hp[:, p0: p0 + w], start=True, stop=True)
                    f = W.tile([H, 2 * CH], BF16, tag="f")
                    nc.scalar.activation(out=f[:, :w], in_=psf[:, :w],
                                         func=ActF.Sigmoid, bias=bftd)
                    fc = W.tile([H, CH], BF16, tag="fcl")
                    nc.vector.tensor_mul(fc[:, :w], f[:, :w], cp[:, p0: p0 + w])
                    for side in range(2):
                        coff = 2 * p0 + side * w
                        iou = PI.tile([H, 3 * CH], F32, tag="iou")
                        for k in range(3):
                            nc.tensor.matmul(iou[:, k * G: k * G + w],
                                             wtdx[:, k * H:(k + 1) * H],
                                             xt[:, xoff + coff: xoff + coff + w],
                                             start=(k * G) % 512 == 0,
                                             stop=False)
                        for k in range(3):
                            nc.tensor.matmul(iou[:, k * G: k * G + w],
                                             wtdh[:, k * H:(k + 1) * H],
                                             hbu[:, xoff + coff: xoff + coff + w],
                                             start=False, stop=False)
                        for k in range(3):
                            nc.tensor.matmul(iou[:, k * G: k * G + w],
                                             utd[:, k * H:(k + 1) * H],
                                             hp[:, p0: p0 + w],
                                             start=False, stop=True)
                        sio = W.tile([H, 2 * CH], BF16, tag="sio")
                        nc.scalar.activation(out=sio[:, :G + w],
                                             in_=iou[:, 0:G + w],
                                             func=ActF.Sigmoid)
                        tu = W.tile([H, CH], BF16, tag="tu")
                        nc.scalar.activation(out=tu[:, :w],
                                             in_=iou[:, 2 * G:2 * G + w],
                                             func=ActF.Tanh)
                        t1 = W.tile([H, CH], BF16, tag="t1")
                        nc.vector.tensor_mul(t1[:, :w], sio[:, :w], tu[:, :w])
                        nc.vector.tensor_add(cw[:, coff: coff + w],
                                             t1[:, :w], fc[:, :w])
                        if pending is not None:
                            td_phase2(pending)
                        pending = (t, coff, w, G, sio)
                if n <= CH:
                    td_phase2(pending)
                    pending = None
            if pending is not None:
                td_phase2(pending)

            # leaf mean: sum the 16 partials, scale by 1/256
            nc.vector.reduce_sum(
                out=outsb[:, 32:64],
                in_=slots.rearrange("p (k t) -> p t k", t=32),
                axis=mybir.AxisListType.X)
            nc.vector.tensor_scalar_mul(outsb[:, 32:64], outsb[:, 32:64],
                                        1.0 / 256.0)
            nc.sync.dma_start(out=OUT[:], in_=outsb[:])

    _split_multi_waits(nc)
    return nc


def _perm():
    """Per-core node permutation: level-grouped chunk-local-split order.
    Entry = row index into the core's [16352, 128] X slab."""
    trees = np.arange(TPC, dtype=np.int64)
    heap = [np.zeros(TPC, dtype=np.int64)]
    tree = [trees.copy()]
    for t in range(1, DEPTH + 1):
        ph, pt = heap[t - 1], tree[t - 1]
        nh, ntr = [], []
        for j in range(0, len(ph), CH):
            bh = ph[j:j + CH]
            bt = pt[j:j + CH]
            nh.append(2 * bh + 1)
            nh.append(2 * bh + 2)
            ntr.append(bt)
            ntr.append(bt)
        heap.append(np.concatenate(nh))
        tree.append(np.concatenate(ntr))
    parts = [tree[t] * M + heap[t] for t in range(DEPTH, -1, -1)]
    return np.concatenate(parts)


def kernel(**inputs):
    from concourse.bass_utils import run_bass_kernel_spmd

    X = np.asarray(inputs["X"], dtype=np.float32)
    W_iou_bu = np.asarray(inputs["W_iou_bu"], dtype=np.float32)
    U_iou_bu = np.asarray(inputs["U_iou_bu"], dtype=np.float32)
    Uf_bu = np.asarray(inputs["Uf_bu"], dtype=np.float32)
    bf_bu = np.asarray(inputs["bf_bu"], dtype=np.float32)
    W_iou_td = np.asarray(inputs["W_iou_td"], dtype=np.float32)
    U_iou_td = np.asarray(inputs["U_iou_td"], dtype=np.float32)
    Uf_td = np.asarray(inputs["Uf_td"], dtype=np.float32)
    bf_td = np.asarray(inputs["bf_td"], dtype=np.float32)

    bf16 = ml_dtypes.bfloat16
    wpack = np.concatenate([
        W_iou_bu.T, U_iou_bu.T, Uf_bu.T,
        W_iou_td[:, :H].T, W_iou_td[:, H:].T, U_iou_td.T, Uf_td.T,
    ], axis=1)
    bpack = np.stack([bf_bu, bf_td], axis=1)
    shared = {
        "WPACK": np.ascontiguousarray(wpack).astype(bf16),
        "BPACK": np.ascontiguousarray(bpack, dtype=np.float32),
    }
    perm = _perm()
    in_maps = []
    for c in range(NCORES):
        slab = X[c * NC_NODES:(c + 1) * NC_NODES]
        xtc = np.ascontiguousarray(slab[perm].T.astype(bf16))
        m = dict(shared)
        m["XT"] = xtc
        in_maps.append(m)

    if "nc" not in _NC_CACHE:
        _NC_CACHE["nc"] = _build_nc()
    nc = _NC_CACHE["nc"]

    trace = bool(os.environ.get("BIDI_TRACE"))
    if trace:
        sys.path.insert(0, "/root/problem/work")
        try:
            import ntff_hook
            ntff_hook.install()
        except Exception:
            trace = False
    res = run_bass_kernel_spmd(nc, in_maps, core_ids=list(range(NCORES)),
                               trace=trace)
    global LAST_EXEC_NS, LAST_TRACE
    LAST_EXEC_NS = res.exec_time_ns
    LAST_TRACE = res.instructions_and_trace

    out = np.empty((B, 2 * H), dtype=np.float32)
    for c in range(NCORES):
        o = res.results[c]["OUT"]          # [128, 64]
        out[c * TPC:(c + 1) * TPC, :H] = o[:, 0:32].T
        out[c * TPC:(c + 1) * TPC, H:] = o[:, 32:64].T
    return out


LAST_EXEC_NS = None
LAST_TRACE = None



# revision 3
# speedup vs baseline: 1.1919x; 1.1919x over previous
"""BiDiTreeLSTM forest kernel for 8 Trainium2 NeuronCores.

Strategy (data-parallel over trees, per the sharding hint):
  - 256 complete binary trees (depth 8, 511 nodes); 32 trees per core.
  - Everything on-device is FEATURE-MAJOR: [128 features on partitions,
    nodes on the free axis].  H == X == 128 exactly fills the partitions.
  - Host pre-permutes each core's nodes into a level-grouped "chunk-local
    split" order: level blocks stored t=8..0; within a level, the children
    of the parents in 512-column chunk j form two adjacent 512-column
    chunks 2j (left) and 2j+1 (right).  Every child/parent gather in both
    propagation passes is then a contiguous column slice, dependencies
    between levels are chunk-local, and tree id == column mod 32 at every
    level (512 % 32 == 0).
  - Bottom-up then top-down level-synchronous ChildSum TreeLSTM per core,
    matmuls in bf16 (fp32 PSUM accumulate), elementwise in bf16 on the
    vector engine (2x/4x mode), gates on the scalar engine.
  - The scalar (ACT) engine is the bottleneck (~330ns fixed cost per
    instruction + ~1ns/col), so ACT instructions are merged aggressively:
      * tanh(u) = 2*sigmoid(2u) - 1 with the u-row blocks of W/U pre-scaled
        by 2 on the host, so i,o,u share ONE sigmoid over the contiguous
        PSUM iou tile; a 1-instruction vector affine (2x-1) recovers tanh.
      * tanh(c)/h of two adjacent chunks are deferred and flushed as ONE
        double-width tanh ("phase2 pairing").
      * top-down forget gates of two adjacent parent chunks share one
        sigmoid and one f*c multiply.
  - Leaf-mean reductions run on the (otherwise idle) gpsimd engine.
  - h0/c0 are zeros and b_iou_* are zeros by problem spec; the kernel
    exploits that (they do not affect the output otherwise).
Output per core: [128, 64] fp32 = [root h_bu | leaf-mean h_td] feature-major;
host reassembles the [256, 256] result.
"""

import os
import sys

sys.path.insert(0, "/opt/trn_rl_repo")

import ml_dtypes
import numpy as np

import concourse.bass as bass
import concourse.mybir as mybir
import concourse.tile as tile

B = 256
DEPTH = 8
M = 511
H = 128
NCORES = 8
TPC = B // NCORES            # trees per core = 32
NC_NODES = TPC * M           # 16352
CH = 512                     # moving-dim chunk (one fp32 PSUM bank)

NT = {t: TPC * (1 << t) for t in range(DEPTH + 1)}      # cols per level
OFF = {}
_o = 0
for _t in range(DEPTH, -1, -1):                           # level 8 first
    OFF[_t] = _o
    _o += NT[_t]
assert _o == NC_NODES

F32 = mybir.dt.float32
BF16 = mybir.dt.bfloat16
ActF = mybir.ActivationFunctionType
Alu = mybir.AluOpType

_NC_CACHE = {}


def _split_multi_waits(nc):
    """This container's walrus accepts at most ONE sync wait per
    instruction; Tile attaches several.  Insert single-wait NoOps."""
    n = 0
    for fn in nc.m.functions:
        for bb in fn.blocks:
            insts = bb.instructions
            new = []
            for inst in insts:
                si = inst.sync_info
                if si is not None and si.on_wait and len(si.on_wait) > 1:
                    waits = list(si.on_wait)
                    for j, w in enumerate(waits[:-1]):
                        new.append(mybir.InstNoOp(
                            name=f"{inst.name}_w{j}",
                            sync_info=mybir.SyncInfo(on_wait=[w], on_update=[]),
                            bass_nofuse=True,
                            engine=inst.engine,
                        ))
                        n += 1
                    si.on_wait = [waits[-1]]
                new.append(inst)
            if len(new) != len(insts):
                bb.instructions[:] = new
    return n


def _build_nc():
    nc = bass.Bass("TRN2")

    XT = nc.dram_tensor("XT", [H, NC_NODES], BF16, kind="ExternalInput")
    # all weight matrices packed: [WBU | UBU | UFBU | WTDX | WTDH | UTD | UFTD]
    # (u-gate row blocks pre-scaled by 2 on the host: tanh(u) = 2*sig(2u)-1)
    WPACK = nc.dram_tensor("WPACK", [H, 2176], BF16, kind="ExternalInput")
    BPACK = nc.dram_tensor("BPACK", [H, 2], F32, kind="ExternalInput")
    OUT = nc.dram_tensor("OUT", [H, 64], F32, kind="ExternalOutput")

    with tile.TileContext(nc) as tc:
        with tc.tile_pool(name="persist", bufs=1) as P, \
             tc.tile_pool(name="ws", bufs=3) as W, \
             tc.tile_pool(name="psiou", bufs=2, space="PSUM") as PI, \
             tc.tile_pool(name="psf", bufs=1, space="PSUM") as PF:

            # ---- persistent SBUF ----
            xt = P.tile([H, NC_NODES], BF16)
            wpk = P.tile([H, 2176], BF16)
            bpk = P.tile([H, 2], F32)
            # xt streamed on the SWDGE (gpsimd) queue so its issue cost
            # overlaps the sync-queue weight loads; graded chunks so level-8
            # compute starts as soon as the first columns land
            nc.gpsimd.dma_start(out=xt[:, 0:512], in_=XT[:, 0:512])
            nc.sync.dma_start(out=wpk[:], in_=WPACK[:])
            nc.sync.dma_start(out=bpk[:], in_=BPACK[:])
            for a, b in ((512, 1536), (1536, 4096), (4096, 10240),
                         (10240, NC_NODES)):
                nc.gpsimd.dma_start(out=xt[:, a:b], in_=XT[:, a:b])
            wbu = wpk[:, 0:384]
            ubu = wpk[:, 384:768]
            ufbu = wpk[:, 768:896]
            wtdx = wpk[:, 896:1280]
            wtdh = wpk[:, 1280:1664]
            utd = wpk[:, 1664:2048]
            uftd = wpk[:, 2048:2176]
            bfbu = bpk[:, 0:1]
            bftd = bpk[:, 1:2]

            hbu = P.tile([H, NC_NODES], BF16)     # all bottom-up h
            ca = P.tile([H, 8192], BF16)          # c ping (even levels)
            cb = P.tile([H, 4096], BF16)          # c pong (odd levels)
            ha = P.tile([H, 2048], BF16)          # td h ping (even, t<8)
            hb = P.tile([H, 4096], BF16)          # td h pong (odd)
            slots = P.tile([H, 256], F32)         # leaf-mean partials
            outsb = P.tile([H, 64], F32)

            def cbuf(t):
                return ca if t % 2 == 0 else cb

            def hbuf(t):
                return ha if t % 2 == 0 else hb

            # ---------------- bottom-up ----------------
            # phase2 (tanh(c), h) is deferred; adjacent same-level chunks
            # flush as a PAIR sharing one double-width tanh.  With the
            # chunk-local layout, chunks later in program order never need
            # the h of the 2 chunks immediately preceding, so a 2-deep
            # deferral is safe within levels with > 2 chunks; levels with
            # <= 2 chunks flush at the level boundary.
            def bu_flush(pend):
                if len(pend) == 2 and pend[0][0] == pend[1][0] \
                        and pend[1][1] == pend[0][1] + pend[0][2]:
                    (t, c0, w, G, sa), (_, _, _, _, sb) = pend
                    tch = W.tile([H, 2 * CH], BF16, tag="tc")
                    nc.scalar.activation(out=tch[:, :2 * w],
                                         in_=cbuf(t)[:, c0: c0 + 2 * w],
                                         func=ActF.Tanh)
                    o = OFF[t] + c0
                    nc.vector.tensor_mul(hbu[:, o: o + w],
                                         sa[:, G:G + w], tch[:, :w])
                    nc.vector.tensor_mul(hbu[:, o + w: o + 2 * w],
                                         sb[:, G:G + w], tch[:, w:2 * w])
                else:
                    for t, c0, w, G, sa in pend:
                        tch = W.tile([H, 2 * CH], BF16, tag="tc")
                        nc.scalar.activation(out=tch[:, :w],
                                             in_=cbuf(t)[:, c0: c0 + w],
                                             func=ActF.Tanh)
                        nc.vector.tensor_mul(
                            hbu[:, OFF[t] + c0: OFF[t] + c0 + w],
                            sa[:, G:G + w], tch[:, :w])
                        if t == 0:
                            nc.vector.tensor_mul(outsb[:, 0:32],
                                                 sa[:, G:G + w], tch[:, :w])
                pend.clear()

            pending = []
            for t in range(DEPTH, -1, -1):
                n = NT[t]
                xoff = OFF[t]
                cw = cbuf(t)
                for c0 in range(0, n, CH):
                    if len(pending) == 2:
                        bu_flush(pending)
                    w = min(CH, n - c0)
                    # gate k lives at col k*G; start=True exactly on the
                    # first matmul touching each PSUM bank (start clears
                    # bank-wide has_written bits)
                    G = w if w < CH else CH
                    iou = PI.tile([H, 3 * CH], F32, tag="iou")
                    for k in range(3):
                        nc.tensor.matmul(
                            iou[:, k * G: k * G + w],
                            wbu[:, k * H:(k + 1) * H],
                            xt[:, xoff + c0: xoff + c0 + w],
                            start=(k * G) % 512 == 0, stop=(t == DEPTH))
                    if t < DEPTH:
                        # children of parents [c0:c0+w] are the adjacent
                        # chunks [2c0 : 2c0+w] and [2c0+w : 2c0+2w]
                        choff = OFF[t + 1]
                        hl = hbu[:, choff + 2 * c0: choff + 2 * c0 + w]
                        hr = hbu[:, choff + 2 * c0 + w: choff + 2 * c0 + 2 * w]
                        cc = cbuf(t + 1)
                        psf = PF.tile([H, 2 * CH], F32, tag="pf")
                        nc.tensor.matmul(psf[:, 0:w], ufbu, hl,
                                         start=True, stop=True)
                        nc.tensor.matmul(psf[:, G:G + w], ufbu, hr,
                                         start=G % 512 == 0, stop=True)
                        if w == n:
                            # single-chunk level: U@(hl+hr) as two matmuls
                            # (skips the htild hop on the critical chain)
                            for k in range(3):
                                nc.tensor.matmul(
                                    iou[:, k * G: k * G + w],
                                    ubu[:, k * H:(k + 1) * H],
                                    hl, start=False, stop=False)
                                nc.tensor.matmul(
                                    iou[:, k * G: k * G + w],
                                    ubu[:, k * H:(k + 1) * H],
                                    hr, start=False, stop=True)
                        else:
                            htild = W.tile([H, CH], BF16, tag="htild")
                            nc.vector.tensor_add(htild[:, :w], hl, hr)
                            for k in range(3):
                                nc.tensor.matmul(
                                    iou[:, k * G: k * G + w],
                                    ubu[:, k * H:(k + 1) * H],
                                    htild[:, :w],
                                    start=False, stop=True)
                        f = W.tile([H, 2 * CH], BF16, tag="f")
                        nc.scalar.activation(out=f[:, :G + w],
                                             in_=psf[:, :G + w],
                                             func=ActF.Sigmoid, bias=bfbu)
                        # fc over both (contiguous) children, then fold
                        fc2 = W.tile([H, 2 * CH], BF16, tag="fc2")
                        nc.vector.tensor_mul(fc2[:, :2 * w], f[:, :2 * w],
                                             cc[:, 2 * c0: 2 * c0 + 2 * w])
                        cred = W.tile([H, CH], BF16, tag="cred")
                        nc.vector.tensor_add(cred[:, :w], fc2[:, :w],
                                             fc2[:, w:2 * w])
                    # ONE sigmoid over [i | o | 2u] (u rows pre-scaled x2)
                    sio = W.tile([H, 3 * CH], BF16, tag="sio")
                    nc.scalar.activation(out=sio[:, :2 * G + w],
                                         in_=iou[:, 0:2 * G + w],
                                         func=ActF.Sigmoid)
                    # tanh(u) = 2*sig(2u) - 1
                    tud = W.tile([H, CH], BF16, tag="tud")
                    nc.vector.tensor_scalar(
                        out=tud[:, :w], in0=sio[:, 2 * G:2 * G + w],
                        scalar1=2.0, scalar2=-1.0,
                        op0=Alu.mult, op1=Alu.add)
                    cdst = cw[:, c0: c0 + w]
                    if t < DEPTH:
                        t1 = W.tile([H, CH], BF16, tag="t1")
                        nc.vector.tensor_mul(t1[:, :w], sio[:, :w], tud[:, :w])
                        nc.vector.tensor_add(cdst, t1[:, :w], cred[:, :w])
                    else:
                        nc.vector.tensor_mul(cdst, sio[:, :w], tud[:, :w])
                    pending.append((t, c0, w, G, sio))
                if n <= 2 * CH:
                    # small level: the next level's first chunk reads this
                    # level's last chunks -- deferral would skip the RAW dep
                    bu_flush(pending)
            if pending:
                bu_flush(pending)

            # ---------------- top-down ----------------
            def td_flush(pend):
                paired = (len(pend) == 2 and pend[0][0] == pend[1][0]
                          and pend[1][1] == pend[0][1] + pend[0][2])
                if paired:
                    (t, coff, w, G, sa), (_, _, _, _, sb) = pend
                    tch = W.tile([H, 2 * CH], BF16, tag="tc")
                    nc.scalar.activation(out=tch[:, :2 * w],
                                         in_=cbuf(t)[:, coff: coff + 2 * w],
                                         func=ActF.Tanh)
                    if t < DEPTH:
                        hw_ = hbuf(t)
                        nc.vector.tensor_mul(hw_[:, coff: coff + w],
                                             sa[:, G:G + w], tch[:, :w])
                        nc.vector.tensor_mul(hw_[:, coff + w: coff + 2 * w],
                                             sb[:, G:G + w], tch[:, w:2 * w])
                    else:
                        hn = W.tile([H, 2 * CH], BF16, tag="hn")
                        nc.vector.tensor_mul(hn[:, :w],
                                             sa[:, G:G + w], tch[:, :w])
                        nc.vector.tensor_mul(hn[:, w:2 * w],
                                             sb[:, G:G + w], tch[:, w:2 * w])
                        pi = coff // (2 * CH)
                        nc.vector.reduce_sum(
                            out=slots[:, pi * 32:(pi + 1) * 32],
                            in_=hn[:, :2 * w].rearrange(
                                "p (k t) -> p t k", t=32),
                            axis=mybir.AxisListType.X)
                else:
                    for t, coff, w, G, sa in pend:
                        tch = W.tile([H, 2 * CH], BF16, tag="tc")
                        nc.scalar.activation(out=tch[:, :w],
                                             in_=cbuf(t)[:, coff: coff + w],
                                             func=ActF.Tanh)
                        nc.vector.tensor_mul(hbuf(t)[:, coff: coff + w],
                                             sa[:, G:G + w], tch[:, :w])
                pend.clear()

            for t in range(0, DEPTH + 1):
                n = NT[t]
                xoff = OFF[t]
                cw = cbuf(t)
                if t == 0:
                    w = n  # 32
                    G = w
                    iou = PI.tile([H, 3 * CH], F32, tag="iou")
                    for k in range(3):
                        nc.tensor.matmul(iou[:, k * G: k * G + w],
                                         wtdx[:, k * H:(k + 1) * H],
                                         xt[:, xoff: xoff + w],
                                         start=(k == 0), stop=False)
                        nc.tensor.matmul(iou[:, k * G: k * G + w],
                                         wtdh[:, k * H:(k + 1) * H],
                                         hbu[:, xoff: xoff + w],
                                         start=False, stop=True)
                    sio = W.tile([H, 3 * CH], BF16, tag="sio")
                    nc.scalar.activation(out=sio[:, :3 * w],
                                         in_=iou[:, 0:3 * w],
                                         func=ActF.Sigmoid)
                    tud = W.tile([H, CH], BF16, tag="tud")
                    nc.vector.tensor_scalar(
                        out=tud[:, :w], in0=sio[:, 2 * G:2 * G + w],
                        scalar1=2.0, scalar2=-1.0,
                        op0=Alu.mult, op1=Alu.add)
                    nc.vector.tensor_mul(cw[:, 0:w], sio[:, :w], tud[:, :w])
                    pending.append((0, 0, w, G, sio))
                    td_flush(pending)
                    continue
                half = n // 2
                hp = hbuf(t - 1)
                cp = cbuf(t - 1)
                fc2td = None
                for p0 in range(0, half, CH):
                    if len(pending) == 2:
                        td_flush(pending)
                    w = min(CH, half - p0)
                    G = w if w < CH else CH
                    if p0 % (2 * CH) == 0:
                        # forget gates for parent chunks p0 and p0+CH share
                        # one sigmoid and one f*c multiply
                        fw = min(2 * CH, half - p0)
                        psf = PF.tile([H, 2 * CH], F32, tag="pf")
                        w1 = min(CH, half - p0)
                        nc.tensor.matmul(psf[:, 0:w1], uftd,
                                         hp[:, p0: p0 + w1],
                                         start=True, stop=True)
                        if fw > CH:
                            nc.tensor.matmul(psf[:, CH:fw], uftd,
                                             hp[:, p0 + CH: p0 + fw],
                                             start=True, stop=True)
                        f = W.tile([H, 2 * CH], BF16, tag="f")
                        nc.scalar.activation(out=f[:, :fw], in_=psf[:, :fw],
                                             func=ActF.Sigmoid, bias=bftd)
                        fc2td = W.tile([H, 2 * CH], BF16, tag="fcl")
                        nc.vector.tensor_mul(fc2td[:, :fw], f[:, :fw],
                                             cp[:, p0: p0 + fw])
                    fc = fc2td[:, (p0 % (2 * CH)):(p0 % (2 * CH)) + w]
                    for side in range(2):
                        coff = 2 * p0 + side * w
                        iou = PI.tile([H, 3 * CH], F32, tag="iou")
                        for k in range(3):
                            nc.tensor.matmul(iou[:, k * G: k * G + w],
                                             wtdx[:, k * H:(k + 1) * H],
                                             xt[:, xoff + coff: xoff + coff + w],
                                             start=(k * G) % 512 == 0,
                                             stop=False)
                        for k in range(3):
                            nc.tensor.matmul(iou[:, k * G: k * G + w],
                                             wtdh[:, k * H:(k + 1) * H],
                                             hbu[:, xoff + coff: xoff + coff + w],
                                             start=False, stop=False)
                        for k in range(3):
                            nc.tensor.matmul(iou[:, k * G: k * G + w],
                                             utd[:, k * H:(k + 1) * H],
                                             hp[:, p0: p0 + w],
                                             start=False, stop=True)
                        sio = W.tile([H, 3 * CH], BF16, tag="sio")
                        nc.scalar.activation(out=sio[:, :2 * G + w],
                                             in_=iou[:, 0:2 * G + w],
                                             func=ActF.Sigmoid)
                        tud = W.tile([H, CH], BF16, tag="tud")
                        nc.vector.tensor_scalar(
                            out=tud[:, :w], in0=sio[:, 2 * G:2 * G + w],
                            scalar1=2.0, scalar2=-1.0,
                            op0=Alu.mult, op1=Alu.add)
                        t1 = W.tile([H, CH], BF16, tag="t1")
                        nc.vector.tensor_mul(t1[:, :w], sio[:, :w],
                                             tud[:, :w])
                        nc.vector.tensor_add(cw[:, coff: coff + w],
                                             t1[:, :w], fc)
                        pending.append((t, coff, w, G, sio))
                if n <= 2 * CH:
                    td_flush(pending)
            if pending:
                td_flush(pending)

            # leaf mean: sum the 8 partials, scale by 1/256
            nc.vector.reduce_sum(
                out=outsb[:, 32:64],
                in_=slots.rearrange("p (k t) -> p t k", t=32),
                axis=mybir.AxisListType.X)
            nc.vector.tensor_scalar_mul(outsb[:, 32:64], outsb[:, 32:64],
                                        1.0 / 256.0)
            nc.sync.dma_start(out=OUT[:], in_=outsb[:])

    _split_multi_waits(nc)
    return nc


def _perm():
    """Per-core node permutation: level-grouped chunk-local-split order.
    Entry = row index into the core's [16352, 128] X slab."""
    trees = np.arange(TPC, dtype=np.int64)
    heap = [np.zeros(TPC, dtype=np.int64)]
    tree = [trees.copy()]
    for t in range(1, DEPTH + 1):
        ph, pt = heap[t - 1], tree[t - 1]
        nh, ntr = [], []
        for j in range(0, len(ph), CH):
            bh = ph[j:j + CH]
            bt = pt[j:j + CH]
            nh.append(2 * bh + 1)
            nh.append(2 * bh + 2)
            ntr.append(bt)
            ntr.append(bt)
        heap.append(np.concatenate(nh))
        tree.append(np.concatenate(ntr))
    parts = [tree[t] * M + heap[t] for t in range(DEPTH, -1, -1)]
    return np.concatenate(parts)


def kernel(**inputs):
    from concourse.bass_utils import run_bass_kernel_spmd

    X = np.asarray(inputs["X"], dtype=np.float32)
    W_iou_bu = np.asarray(inputs["W_iou_bu"], dtype=np.float32)
    U_iou_bu = np.asarray(inputs["U_iou_bu"], dtype=np.float32)
    Uf_bu = np.asarray(inputs["Uf_bu"], dtype=np.float32)
    bf_bu = np.asarray(inputs["bf_bu"], dtype=np.float32)
    W_iou_td = np.asarray(inputs["W_iou_td"], dtype=np.float32)
    U_iou_td = np.asarray(inputs["U_iou_td"], dtype=np.float32)
    Uf_td = np.asarray(inputs["Uf_td"], dtype=np.float32)
    bf_td = np.asarray(inputs["bf_td"], dtype=np.float32)

    bf16 = ml_dtypes.bfloat16
    # u-gate row blocks scaled by 2: the kernel computes
    # tanh(u) = 2*sigmoid(2u) - 1 on the vector engine
    s2 = np.ones((1, 3 * H), np.float32)
    s2[:, 2 * H:] = 2.0
    wpack = np.concatenate([
        W_iou_bu.T * s2, U_iou_bu.T * s2, Uf_bu.T,
        W_iou_td[:, :H].T * s2, W_iou_td[:, H:].T * s2, U_iou_td.T * s2,
        Uf_td.T,
    ], axis=1)
    bpack = np.stack([bf_bu, bf_td], axis=1)
    shared = {
        "WPACK": np.ascontiguousarray(wpack).astype(bf16),
        "BPACK": np.ascontiguousarray(bpack, dtype=np.float32),
    }
    perm = _perm()
    in_maps = []
    for c in range(NCORES):
        slab = X[c * NC_NODES:(c + 1) * NC_NODES]
        xtc = np.ascontiguousarray(slab[perm].T.astype(bf16))
        m = dict(shared)
        m["XT"] = xtc
        in_maps.append(m)

    if "nc" not in _NC_CACHE:
        _NC_CACHE["nc"] = _build_nc()
    nc = _NC_CACHE["nc"]

    trace = bool(os.environ.get("BIDI_TRACE"))
    if trace:
        sys.path.insert(0, "/root/problem/work")
        try:
            import ntff_hook
            ntff_hook.install()
        except Exception:
            trace = False
    res = run_bass_kernel_spmd(nc, in_maps, core_ids=list(range(NCORES)),
                               trace=trace)
    global LAST_EXEC_NS, LAST_TRACE
    LAST_EXEC_NS = res.exec_time_ns
    LAST_TRACE = res.instructions_and_trace

    out = np.empty((B, 2 * H), dtype=np.float32)
    for c in range(NCORES):
        o = res.results[c]["OUT"]          # [128, 64]
        out[c * TPC:(c + 1) * TPC, :H] = o[:, 0:32].T
        out[c * TPC:(c + 1) * TPC, H:] = o[:, 32:64].T
    return out


LAST_EXEC_NS = None
LAST_TRACE = None
